# revision 1
# baseline (speedup 1.0000x reference)
"""Black-Scholes 'all' pricing on 8 Trainium2 NeuronCores (Bass/Tile).

kernel(S0, K, T, vt) -> [N, 4] float32 (call, put, digital_call, digital_put)
N = 8_388_608; options sharded contiguously across 8 cores, each core
processing 1M elements as [128 partitions x 8192] in tiles of F=1024.

Design, from measured HW behavior (microbenchmarks + kernel traces):
- DVE rules: f32/mixed TENSOR_TENSOR = 1 elem/cycle (~1215ns per [128,1024]
  op); all-f16 packed TT = 2x (~685ns); custom DVE ops and STT are always
  1x and custom f16 *output* costs ~+700ns; strided SBUF writes ~+750ns+;
  reading BOTH operands from the same SBUF tile/pool region causes bank
  conflicts (~1.4-2.8x); a PSUM *read* is fine if the other operand is
  SBUF, and DVE/ACT writes to PSUM are free.
- Placement: every two-input DVE op gets operands from different memories
  where possible: dq, lnKr, isv and t1 live in PSUM (ring-1, 16KB total),
  so Sq=s*dq, b=lnSq-lnKr, d1/d2=numer*isv, call=t1-t2 all read
  PSUM+SBUF. numer's b operand sits in a distant SBUF pool (bank dodge).
- d2 = (b - 0.5*vt*T)*isv instead of d1 - sv: kills the sv=exp(u/2) ACT
  op and makes d1/d2 two independent muls against PSUM isv.
- Outputs are four CONTIGUOUS planes (call/put f16, digitals f32 - f16
  custom-out is slow, TT converts to f16 for free). No on-chip [N,4]
  interleave (the old stride-4 writes cost ~32us); the host stacks the
  planes during the unshard. put = call + pc runs all-f16 at 2x.
- GPSIMD holds exactly 3 full-width ops (vtt, pc->f16, numer2 = numer-vtt):
  4+ gpsimd ops hammer the shared SBUF port and inflate every DVE op by
  ~50% (measured).
- ACT: ~1148ns per f32 [128,1024] op regardless of function; erf runs on
  the [d1|d2] f16 pair in one rank-3 op. ln MUST come from `natural_log`
  (the combined set's ln is ~16x less accurate; its error is amplified by
  isv=1/sqrt(vt*T) up to 100x and lands in the digital outputs). exp:
  `exp_and_others`, erf: `sigmoid_and_others`. ACT work is batched per
  table set in sub-phases over groups of G tiles with explicit
  same-engine dep edges (13 ACT_TABLE_LOADs total).
- Input DMAs issue t,v before s,k (t feeds the dq/dr/vtt critical path).
  Group schedule: single-tile front group (primes the pipeline ~3us
  sooner), full G=2 groups in steady state, single-tile end group
  (smaller drain). ep (erf output) is ring-2 so ACT erf never waits for
  the previous tile's DVE tail to free its slot.
- Known engine budget per core: DVE ~121us (the wall), ACT ~89us,
  GPSIMD ~66us, DMA 28MiB ~82us. HW exec ~154us at base clock
  (vs 177us baseline); the device shows ~20% DVFS run-to-run variance.
"""
import numpy as np

import concourse.bass as bass
import concourse.tile as tile
from concourse import bacc, mybir
from concourse.bass_utils import run_bass_kernel_spmd
from concourse.dve_ops import AFFINE_MUL_REDUCE
from concourse.tile_rust import add_dep_helper

F32 = mybir.dt.float32
F16 = mybir.dt.float16
AF = mybir.ActivationFunctionType
OP = mybir.AluOpType

R = 0.02
Q = 0.01
INV_SQRT2 = 0.7071067811865476

N = 8_388_608
NCORES = 8
P = 128
FD = N // NCORES // P  # 8192

_KEEP_SETS = ("exp_and_others", "sigmoid_and_others", "natural_log")
_orig_get_tables = None

_NC = None
LAST_EXEC_NS = None
LAST_TRACE_DIR = None
TRACE = False


def _patch_act_tables():
    """Blank the membership of every activation-table set except the three
    we use (list order preserved, so act_func_set_id indices into
    act_info.json stay valid) so the table-load pass resolves ln/exp/erf
    to the sets we want."""
    global _orig_get_tables
    import concourse.hw_specs as hw_specs
    if _orig_get_tables is None:
        _orig_get_tables = hw_specs.get_activation_tables

        def patched(arch):
            tabs = _orig_get_tables(arch)
            return {
                name: (fns if name in _KEEP_SETS else set())
                for name, fns in tabs.items()
            }

        hw_specs.get_activation_tables = patched
        bacc.get_activation_tables = patched


def build_bs(FD=FD, F=1024, G=2, P=P):
    from contextlib import ExitStack
    assert FD % F == 0
    _patch_act_tables()
    ntiles = FD // F
    nc = bacc.Bacc("TRN2", target_bir_lowering=False, debug=False,
                   num_devices=NCORES)
    s_d = nc.dram_tensor("s0", [P, FD], F32, kind="ExternalInput").ap()
    k_d = nc.dram_tensor("k", [P, FD], F32, kind="ExternalInput").ap()
    t_d = nc.dram_tensor("t", [P, FD], F32, kind="ExternalInput").ap()
    v_d = nc.dram_tensor("vt", [P, FD], F32, kind="ExternalInput").ap()
    oc_d = nc.dram_tensor("oc", [P, FD], F16, kind="ExternalOutput").ap()
    op_d = nc.dram_tensor("op", [P, FD], F16, kind="ExternalOutput").ap()
    odc_d = nc.dram_tensor("odc", [P, FD], F32, kind="ExternalOutput").ap()
    odp_d = nc.dram_tensor("odp", [P, FD], F32, kind="ExternalOutput").ap()

    def am(out, in0, in1, s0, s1):
        # out = (in0*s0 + s1) * in1
        nc.vector._custom_dve(AFFINE_MUL_REDUCE, out=out, in0=in0, in1=in1,
                              s0=s0, s1=s1)

    with tile.TileContext(nc) as tc, ExitStack() as ctx:
        inp = ctx.enter_context(tc.tile_pool(name="inp", bufs=2))
        pers = ctx.enter_context(tc.tile_pool(name="pers", bufs=2 * G))
        mida = ctx.enter_context(tc.tile_pool(name="mida", bufs=2))
        midc = ctx.enter_context(tc.tile_pool(name="midc", bufs=2))
        perss = ctx.enter_context(tc.tile_pool(name="perss", bufs=2))
        psA = ctx.enter_context(tc.tile_pool(name="psA", bufs=1, space="PSUM"))
        midb = ctx.enter_context(tc.tile_pool(name="midb", bufs=2))
        outp = ctx.enter_context(tc.tile_pool(name="outp", bufs=2))

        # front-prime with a single tile (fills the pipeline sooner), full
        # G-sized groups in steady state, one single-tile group at the end
        # (smaller drain). Single-tile groups stall more per tile, so keep
        # them only at the edges.
        sizes = [1] + [G] * ((ntiles - 2) // G) + [1]
        assert sum(sizes) == ntiles
        glist = []
        pos = 0
        for size in sizes:
            glist.append(range(pos, pos + size))
            pos += size

        # ACT-stream phase ordering: chain every ACT op of a sub-phase after
        # all ACT ops of the previous sub-phase, so the scheduler cannot
        # interleave different table sets and thrash ACT_TABLE_LOADs.
        prev_phase = []
        cur_phase = []

        def act(*args, **kwargs):
            bi = nc.scalar.activation(*args, **kwargs)
            for p in prev_phase:
                add_dep_helper(bi.ins, p.ins, sync=False,
                               reason="act table phase ordering")
            cur_phase.append(bi)
            return bi

        def end_phase():
            if cur_phase:
                prev_phase[:] = cur_phase
                cur_phase.clear()

        # Warmup: a dependency-free 8-element exp forces the exp-set
        # ACT_TABLE_LOAD to run during the engine preamble / input-DMA
        # window instead of serializing behind the first t-tile arrival
        # (the load binds to the first exp op in the in-order ACT queue).
        warm = mida.tile([P, 8], F32, tag="warm", bufs=1)
        nc.gpsimd.memset(warm[:], 0.0)
        warm2 = mida.tile([P, 8], F32, tag="warm2", bufs=1)
        act(warm2[:], warm[:], AF.Exp)

        st = {}  # per-tile tensor handles

        def emit_sp3(tiles):
            # (exp_and_others): isv, sv; DVE d1, d2 (f32 dpair) — emitted
            # inside the next group's SP1 phase to share one exp residency.
            for i in tiles:
                z = st[i]
                isv = psA.tile([P, F], F32, tag="isv")
                act(isv[:], z["u"][:], AF.Exp, scale=-0.5)
                dpair = perss.tile([P, 2, F], F16, tag="dp")
                nc.vector.tensor_mul(dpair[:, 0], z["numer"][:], isv[:])
                nc.vector.tensor_mul(dpair[:, 1], z["numer2"][:], isv[:])
                z["dpair"] = dpair

        def emit_sp4(tiles):
            # (sigmoid_and_others): one erf over [d1|d2] (f32); tail with a
            # closed f16 island; DMA out 4 contiguous f16 planes.
            for i in tiles:
                z = st.pop(i)
                sl = slice(i * F, (i + 1) * F)
                ep = midb.tile([P, 2, F], F32, tag="ep", bufs=2)
                act(ep[:], z["dpair"][:], AF.Erf, scale=INV_SQRT2)
                t1 = psA.tile([P, F], F32, tag="t1")
                am(t1[:], ep[:, 0], z["sqkr"][:, 0], 0.5, 0.5)
                t2 = midb.tile([P, F], F32, tag="t2", bufs=1)
                am(t2[:], ep[:, 1], z["sqkr"][:, 1], 0.5, 0.5)
                oc = outp.tile([P, F], F16, tag="oc")
                nc.vector.tensor_sub(oc[:], t1[:], t2[:])
                op_ = outp.tile([P, F], F16, tag="op")
                nc.vector.tensor_add(op_[:], oc[:], z["pc"][:])
                odc = outp.tile([P, F], F32, tag="odc")
                am(odc[:], ep[:, 1], z["dr"][:], 0.5, 0.5)
                odp = outp.tile([P, F], F32, tag="odp")
                am(odp[:], ep[:, 1], z["dr"][:], -0.5, 0.5)
                nc.sync.dma_start(oc_d[:, sl], oc[:])
                nc.sync.dma_start(op_d[:, sl], op_[:])
                nc.sync.dma_start(odc_d[:, sl], odc[:])
                nc.sync.dma_start(odp_d[:, sl], odp[:])

        prev_tiles = None
        for tiles in glist:
            # ---- SP1 (exp_and_others): [prev group isv/sv] + dq, dr ----
            if prev_tiles is not None:
                emit_sp3(prev_tiles)
            for i in tiles:
                sl = slice(i * F, (i + 1) * F)
                t = inp.tile([P, F], F32, tag="t", bufs=3)
                nc.sync.dma_start(t[:], t_d[:, sl])
                v = inp.tile([P, F], F32, tag="v", bufs=3)
                nc.sync.dma_start(v[:], v_d[:, sl])
                s = inp.tile([P, F], F32, tag="s")
                nc.sync.dma_start(s[:], s_d[:, sl])
                k = inp.tile([P, F], F32, tag="k")
                nc.sync.dma_start(k[:], k_d[:, sl])

                dq = psA.tile([P, F], F32, tag="dq")
                act(dq[:], t[:], AF.Exp, scale=-Q)
                dr = pers.tile([P, F], F32, tag="dr")
                act(dr[:], t[:], AF.Exp, scale=-R)
                vtt = mida.tile([P, F], F32, tag="vtt")
                nc.gpsimd.tensor_mul(vtt[:], t[:], v[:])
                sqkr = pers.tile([P, 2, F], F32, tag="sqkr")
                nc.vector.tensor_mul(sqkr[:, 0], s[:], dq[:])
                nc.vector.tensor_mul(sqkr[:, 1], k[:], dr[:])
                pc = mida.tile([P, F], F16, tag="pc", bufs=2 * G)
                nc.gpsimd.tensor_sub(pc[:], sqkr[:, 1], sqkr[:, 0])
                st[i] = dict(dr=dr, sqkr=sqkr, vtt=vtt, pc=pc)
            end_phase()
            # ---- erf phase for the previous group ----
            if prev_tiles is not None:
                emit_sp4(prev_tiles)
                end_phase()
            # ---- SP2 (natural_log): ln[Sq|Kr], ln vtt; b, numer ----
            for i in tiles:
                z = st[i]
                lnS = mida.tile([P, F], F32, tag="lnS")
                act(lnS[:], z["sqkr"][:, 0], AF.Ln)
                lnK = psA.tile([P, F], F32, tag="lnK")
                act(lnK[:], z["sqkr"][:, 1], AF.Ln)
                u = midc.tile([P, F], F32, tag="u")
                act(u[:], z["vtt"][:], AF.Ln)
                b = perss.tile([P, F], F32, tag="b")
                nc.vector.tensor_sub(b[:], lnS[:], lnK[:])
                numer = midb.tile([P, F], F32, tag="numer")
                nc.vector.scalar_tensor_tensor(
                    numer[:], z["vtt"][:], 0.5, b[:], OP.mult, OP.add)
                numer2 = midc.tile([P, F], F32, tag="numer2")
                nc.gpsimd.tensor_sub(numer2[:], numer[:], z["vtt"][:])
                z["u"] = u
                z["numer"] = numer
                z["numer2"] = numer2
            end_phase()
            prev_tiles = tiles
        # drain the last group
        emit_sp3(prev_tiles)
        end_phase()
        emit_sp4(prev_tiles)
        end_phase()
    nc.compile()
    return nc


def _get_nc():
    global _NC
    if _NC is None:
        _NC = build_bs()
    return _NC


def kernel(S0, K, T, vt):
    global LAST_EXEC_NS, LAST_TRACE_DIR
    nc = _get_nc()
    arrs = {
        "s0": np.asarray(S0, dtype=np.float32),
        "k": np.asarray(K, dtype=np.float32),
        "t": np.asarray(T, dtype=np.float32),
        "vt": np.asarray(vt, dtype=np.float32),
    }
    shards = []
    for i in range(NCORES):
        sl = slice(i * P * FD, (i + 1) * P * FD)
        shards.append({
            name: np.ascontiguousarray(a[sl].reshape(P, FD))
            for name, a in arrs.items()
        })
    kwargs = {}
    if TRACE:
        import tempfile
        LAST_TRACE_DIR = tempfile.mkdtemp(prefix="bs_trace_")
        kwargs = dict(trace=True, tmpdir=LAST_TRACE_DIR)
    res = run_bass_kernel_spmd(nc, shards, core_ids=list(range(NCORES)),
                               **kwargs)
    LAST_EXEC_NS = res.exec_time_ns
    out = np.empty((N, 4), dtype=np.float32)
    for i in range(NCORES):
        sl = slice(i * P * FD, (i + 1) * P * FD)
        r = res.results[i]
        out[sl, 0] = r["oc"].reshape(-1).astype(np.float32)
        out[sl, 1] = r["op"].reshape(-1).astype(np.float32)
        out[sl, 2] = r["odc"].reshape(-1)
        out[sl, 3] = r["odp"].reshape(-1)
    return out



# revision 2
# speedup vs baseline: 1.1579x; 1.1579x over previous
"""Black-Scholes 'all' pricing on 8 Trainium2 NeuronCores (Bass/Tile).

kernel(S0, K, T, vt) -> [N, 4] float32 (call, put, digital_call, digital_put)
N = 8_388_608; options sharded contiguously across 8 cores, each core
processing 1M elements as [128 partitions x 8192] in tiles of F=1024.

Redesign vs the 153us baseline (measured on HW):
- T and vt ship as f16 from the host (error via isv-amplification is
  sqrt(vtt)-scaled, stays ~3e-4 in d); S0/K stay f32 for the log chain.
  All four outputs are f16 planes. DMA: 28 MiB -> 21 MiB per core.
- ln(S) and ln(K) are taken on RAW inputs with ACT scale=0.01
  (ln(0.01*s) in [-0.23, 0.19]) and written as float32r. The constant
  ln(100) offsets cancel in b = lnS - lnK. The small magnitude matters:
  fp32r rounds to ~12 mantissa bits (measured rel 2.3e-4), so scaling
  keeps the abs error ~5e-5 -> harmless after isv<=100 amplification.
- numer = b + (R-Q)T + 0.5*vtt is computed ON THE PE ENGINE as four
  identity-weight matmuls accumulating in PSUM (fp32r identity matmul
  measured bit-exact; mixed f32r/f16 accumulation groups work). This
  pulls the b/numer adds off the DVE wall. d2 = d1 - sv (sv = vtt*isv
  on GPSIMD) replaces the baseline's numer2 path.
- Since lnS/lnK no longer come from Sq/Kr, the discount factors are
  born f16: dq = exp(-Q*t) f16 from ACT, dr = dq^2 on GPSIMD (R = 2Q).
  The entire output side runs in f16 at DVE 2x (TT) / 4x (tensor_scalar
  for N = 0.5*erf+0.5): t1 = Sq*N1, t2 = Kr*N2, oc = t1-t2, op = oc-pc,
  odc = dr*N2, odp = dr-odc, with pc = Sq-Kr.
- exp AND ln both live in the natural_log_exp_and_others ACT table set
  (its ln measured 1.9e-5 abs err on [80,120] - fine at 2e-2 tolerance),
  so the table cycle is 2 sets (ln/exp + erf) instead of 3: ~5
  ACT_TABLE_LOADs (1283ns each) instead of 13.
- GPSIMD holds exactly 3 full-width ops (vtt, dr, sv) - 4+ measured to
  inflate DVE by ~50% in the previous session.
- Engine budget per core per tile (F=1024): DVE ~9.7us (7.5 f32-units:
  SqKr-pair mixed 2.0, d1 mixed-psum 1.0, d2 0.5, pc 0.5, N-pair 0.5,
  t1/t2/oc/odc/op/odp 0.5 each), ACT ~7.4us (lnSK-pair 2, dq 1, u 1,
  isv 1, erf-pair 2), PE ~3.5us, GPS ~6us. DVE-walled ~78us vs DMA 62us.
"""
import numpy as np

import concourse.bass as bass
import concourse.tile as tile
from concourse import bacc, mybir
from concourse.bass_utils import run_bass_kernel_spmd
from concourse.tile_rust import add_dep_helper

F32 = mybir.dt.float32
F32R = mybir.dt.float32r
F16 = mybir.dt.float16
AF = mybir.ActivationFunctionType
OP = mybir.AluOpType

R = 0.02
Q = 0.01
INV_SQRT2 = 0.7071067811865476
SCL = 0.01  # ln input scale: ln(SCL*s), offsets cancel in b

N = 8_388_608
NCORES = 8
P = 128
FD = N // NCORES // P  # 8192

_KEEP_SETS = ("natural_log_exp_and_others", "sigmoid_and_others")
_orig_get_tables = None

_NC = None
LAST_EXEC_NS = None
LAST_TRACE_DIR = None
TRACE = False


def _patch_act_tables():
    """Blank the membership of every activation-table set except the two
    we use (list order preserved so act_func_set_id indices stay valid)
    so ln/exp resolve to the combined set and erf to sigmoid_and_others."""
    global _orig_get_tables
    import concourse.hw_specs as hw_specs
    if _orig_get_tables is None:
        _orig_get_tables = hw_specs.get_activation_tables

        def patched(arch):
            tabs = _orig_get_tables(arch)
            return {
                name: (fns if name in _KEEP_SETS else set())
                for name, fns in tabs.items()
            }

        hw_specs.get_activation_tables = patched
        bacc.get_activation_tables = patched


def build_bs(FD=FD, F=1024, G=2, P=P):
    from contextlib import ExitStack
    assert FD % F == 0
    _patch_act_tables()
    ntiles = FD // F
    nchunks = F // 512  # matmul moving-operand chunks (psum bank = 512 f32)
    nc = bacc.Bacc("TRN2", target_bir_lowering=False, debug=False,
                   num_devices=NCORES)
    s_d = nc.dram_tensor("s0", [P, FD], F32, kind="ExternalInput").ap()
    k_d = nc.dram_tensor("k", [P, FD], F32, kind="ExternalInput").ap()
    t_d = nc.dram_tensor("t", [P, FD], F16, kind="ExternalInput").ap()
    v_d = nc.dram_tensor("vt", [P, FD], F16, kind="ExternalInput").ap()
    eyeP_d = nc.dram_tensor("eyep", [P, P], F32R, kind="ExternalInput").ap()
    eyeN_d = nc.dram_tensor("eyen", [P, P], F32R, kind="ExternalInput").ap()
    eyeC_d = nc.dram_tensor("eyec", [P, P], F16, kind="ExternalInput").ap()
    eyeH_d = nc.dram_tensor("eyeh", [P, P], F16, kind="ExternalInput").ap()
    oc_d = nc.dram_tensor("oc", [P, FD], F16, kind="ExternalOutput").ap()
    op_d = nc.dram_tensor("op", [P, FD], F16, kind="ExternalOutput").ap()
    odc_d = nc.dram_tensor("odc", [P, FD], F16, kind="ExternalOutput").ap()
    odp_d = nc.dram_tensor("odp", [P, FD], F16, kind="ExternalOutput").ap()

    with tile.TileContext(nc) as tc, ExitStack() as ctx:
        inp = ctx.enter_context(tc.tile_pool(name="inp", bufs=3))
        consts = ctx.enter_context(tc.tile_pool(name="consts", bufs=1))
        mid1 = ctx.enter_context(tc.tile_pool(name="mid1", bufs=2 * G + 1))
        mid2 = ctx.enter_context(tc.tile_pool(name="mid2", bufs=2))
        mid3 = ctx.enter_context(tc.tile_pool(name="mid3", bufs=3))
        psA = ctx.enter_context(tc.tile_pool(name="psA", bufs=2, space="PSUM"))
        mid4 = ctx.enter_context(tc.tile_pool(name="mid4", bufs=2 * G + 1))
        mid5 = ctx.enter_context(tc.tile_pool(name="mid5", bufs=2))
        mid6 = ctx.enter_context(tc.tile_pool(name="mid6", bufs=2))
        outp = ctx.enter_context(tc.tile_pool(name="outp", bufs=2))

        # identity-weight matrices for the PE numer accumulation
        eyeP = consts.tile([P, P], F32R, tag="eyeP")
        nc.sync.dma_start(eyeP[:], eyeP_d)
        eyeN = consts.tile([P, P], F32R, tag="eyeN")
        nc.sync.dma_start(eyeN[:], eyeN_d)
        eyeC = consts.tile([P, P], F16, tag="eyeC")
        nc.sync.dma_start(eyeC[:], eyeC_d)
        eyeH = consts.tile([P, P], F16, tag="eyeH")
        nc.sync.dma_start(eyeH[:], eyeH_d)

        sizes = [1] + [G] * ((ntiles - 2) // G) + [1]
        assert sum(sizes) == ntiles
        glist = []
        pos = 0
        for size in sizes:
            glist.append(range(pos, pos + size))
            pos += size

        # ACT-stream phase ordering: chain every ACT op of a sub-phase after
        # all ACT ops of the previous sub-phase so the scheduler cannot
        # interleave the two table sets and thrash ACT_TABLE_LOADs.
        prev_phase = []
        cur_phase = []

        def act(*args, **kwargs):
            bi = nc.scalar.activation(*args, **kwargs)
            for p in prev_phase:
                add_dep_helper(bi.ins, p.ins, sync=False,
                               reason="act table phase ordering")
            cur_phase.append(bi)
            return bi

        def end_phase():
            if cur_phase:
                prev_phase[:] = cur_phase
                cur_phase.clear()

        # Warmup: dependency-free 8-elem exp forces the ln/exp-set
        # ACT_TABLE_LOAD during the engine preamble / input-DMA window.
        warm = mid3.tile([P, 8], F32, tag="warm", bufs=1)
        nc.gpsimd.memset(warm[:], 0.0)
        warm2 = mid3.tile([P, 8], F32, tag="warm2", bufs=1)
        act(warm2[:], warm[:], AF.Exp)

        st = {}  # per-tile tensor handles

        def emit_tail(tiles):
            # E-phase DVE tail + output DMAs for a finished group
            for i in tiles:
                z = st.pop(i)
                sl = slice(i * F, (i + 1) * F)
                npair = mid5.tile([P, 2, F], F16, tag="npair")
                nc.vector.tensor_scalar(npair[:], z["ep"][:], scalar1=0.5,
                                        scalar2=0.5, op0=OP.mult, op1=OP.add)
                t1 = mid2.tile([P, F], F16, tag="t1")
                nc.vector.tensor_mul(t1[:], z["sq"][:], npair[:, 0])
                t2 = mid6.tile([P, F], F16, tag="t2")
                nc.vector.tensor_mul(t2[:], z["kr"][:], npair[:, 1])
                oc = outp.tile([P, F], F16, tag="oc")
                nc.vector.tensor_sub(oc[:], t1[:], t2[:])
                odc = outp.tile([P, F], F16, tag="odc")
                nc.vector.tensor_mul(odc[:], z["dqdr"][:, 1], npair[:, 1])
                op_ = outp.tile([P, F], F16, tag="op")
                nc.vector.tensor_sub(op_[:], oc[:], z["pc"][:])
                odp = outp.tile([P, F], F16, tag="odp")
                nc.vector.tensor_sub(odp[:], z["dqdr"][:, 1], odc[:])
                nc.sync.dma_start(oc_d[:, sl], oc[:])
                nc.sync.dma_start(op_d[:, sl], op_[:])
                nc.sync.dma_start(odc_d[:, sl], odc[:])
                nc.sync.dma_start(odp_d[:, sl], odp[:])

        prev_tiles = None
        for tiles in glist:
            # ---- L phase (ln/exp set) ----
            for i in tiles:
                sl = slice(i * F, (i + 1) * F)
                t = inp.tile([P, F], F16, tag="t")
                nc.sync.dma_start(t[:], t_d[:, sl])
                v = inp.tile([P, F], F16, tag="v")
                nc.sync.dma_start(v[:], v_d[:, sl])
                sk = inp.tile([P, 2, F], F32, tag="sk")
                nc.sync.dma_start(sk[:, 0], s_d[:, sl])
                nc.sync.dma_start(sk[:, 1], k_d[:, sl])

                dqdr = mid1.tile([P, 2, F], F16, tag="dqdr")
                act(dqdr[:, 0], t[:], AF.Exp, scale=-Q)
                vtt = mid3.tile([P, F], F16, tag="vtt")
                nc.gpsimd.tensor_mul(vtt[:], t[:], v[:])
                lnsk = mid2.tile([P, 2, F], F32R, tag="lnsk")
                act(lnsk[:], sk[:], AF.Ln, scale=SCL)
                u = mid3.tile([P, F], F16, tag="u")
                act(u[:], vtt[:], AF.Ln)
                isv = mid3.tile([P, F], F16, tag="isv")
                act(isv[:], u[:], AF.Exp, scale=-0.5)
                # dr = dq^2 on gpsimd (R = 2Q)
                nc.gpsimd.tensor_mul(dqdr[:, 1], dqdr[:, 0], dqdr[:, 0])
                # sv = sqrt(vtt) = vtt * isv on gpsimd
                sv = mid5.tile([P, F], F16, tag="sv")
                nc.gpsimd.tensor_mul(sv[:], vtt[:], isv[:])

                # numer = lnS - lnK + (R-Q)*t + 0.5*vtt on PE (psum accum)
                numer = psA.tile([P, F], F32, tag="numer")
                for w, src, c0 in (
                    (eyeP, lnsk[:, 0], True),
                    (eyeN, lnsk[:, 1], False),
                    (eyeC, t[:], False),
                    (eyeH, vtt[:], False),
                ):
                    for c in range(nchunks):
                        cs = slice(c * 512, (c + 1) * 512)
                        nc.tensor.matmul(numer[:, cs], w[:], src[:, cs],
                                         start=c0, stop=(w is eyeH),
                                         skip_group_check=True)

                # DVE mid-chain
                sq = mid6.tile([P, F], F16, tag="sq")
                nc.vector.tensor_mul(sq[:], sk[:, 0], dqdr[:, 0])
                kr = mid4.tile([P, F], F16, tag="kr")
                nc.vector.tensor_mul(kr[:], sk[:, 1], dqdr[:, 1])
                pc = mid3.tile([P, F], F16, tag="pc", bufs=2 * G + 1)
                nc.vector.tensor_sub(pc[:], sq[:], kr[:])
                dpair = mid4.tile([P, 2, F], F16, tag="dpair")
                nc.vector.tensor_mul(dpair[:, 0], numer[:], isv[:])
                nc.vector.tensor_sub(dpair[:, 1], dpair[:, 0], sv[:])
                st[i] = dict(dqdr=dqdr, sq=sq, kr=kr, pc=pc, dpair=dpair)
            end_phase()
            # ---- E phase (erf set) for the previous group ----
            if prev_tiles is not None:
                for i in prev_tiles:
                    z = st[i]
                    ep = mid6.tile([P, 2, F], F16, tag="ep")
                    act(ep[:], z["dpair"][:], AF.Erf, scale=INV_SQRT2)
                    z["ep"] = ep
                end_phase()
                emit_tail(prev_tiles)
            prev_tiles = tiles
        # drain the last group
        for i in prev_tiles:
            z = st[i]
            ep = mid6.tile([P, 2, F], F16, tag="ep")
            act(ep[:], z["dpair"][:], AF.Erf, scale=INV_SQRT2)
            z["ep"] = ep
        end_phase()
        emit_tail(prev_tiles)
    nc.compile()
    return nc


def _get_nc():
    global _NC
    if _NC is None:
        _NC = build_bs()
    return _NC


def kernel(S0, K, T, vt):
    global LAST_EXEC_NS, LAST_TRACE_DIR
    nc = _get_nc()
    s32 = np.asarray(S0, dtype=np.float32)
    k32 = np.asarray(K, dtype=np.float32)
    t16 = np.asarray(T, dtype=np.float32).astype(np.float16)
    v16 = np.asarray(vt, dtype=np.float32).astype(np.float16)
    eye = np.eye(P, dtype=np.float32)
    eyep = eye
    eyen = -eye
    eyec = (eye * (R - Q)).astype(np.float16)
    eyeh = (eye * 0.5).astype(np.float16)
    shards = []
    for i in range(NCORES):
        sl = slice(i * P * FD, (i + 1) * P * FD)
        shards.append({
            "s0": np.ascontiguousarray(s32[sl].reshape(P, FD)),
            "k": np.ascontiguousarray(k32[sl].reshape(P, FD)),
            "t": np.ascontiguousarray(t16[sl].reshape(P, FD)),
            "vt": np.ascontiguousarray(v16[sl].reshape(P, FD)),
            "eyep": eyep, "eyen": eyen, "eyec": eyec, "eyeh": eyeh,
        })
    kwargs = {}
    if TRACE:
        import tempfile
        LAST_TRACE_DIR = tempfile.mkdtemp(prefix="bs_trace_")
        kwargs = dict(trace=True, tmpdir=LAST_TRACE_DIR)
    res = run_bass_kernel_spmd(nc, shards, core_ids=list(range(NCORES)),
                               **kwargs)
    LAST_EXEC_NS = res.exec_time_ns
    out = np.empty((N, 4), dtype=np.float32)
    for i in range(NCORES):
        sl = slice(i * P * FD, (i + 1) * P * FD)
        r = res.results[i]
        out[sl, 0] = r["oc"].reshape(-1).astype(np.float32)
        out[sl, 1] = r["op"].reshape(-1).astype(np.float32)
        out[sl, 2] = r["odc"].reshape(-1).astype(np.float32)
        out[sl, 3] = r["odp"].reshape(-1).astype(np.float32)
    return out


# revision 11
# speedup vs baseline: 1.3779x; 1.1900x over previous
"""Black-Scholes 'all' pricing on 8 Trainium2 NeuronCores (Bass/Tile).

kernel(S0, K, T, vt) -> [N, 4] float32 (call, put, digital_call, digital_put)
N = 8_388_608; options sharded contiguously across 8 cores, each core
processing 1M elements as [128 partitions x 8192] in tiles of F=1024.

Redesign vs the 153us baseline (measured on HW):
- T and vt ship as f16 from the host (error via isv-amplification is
  sqrt(vtt)-scaled, stays ~3e-4 in d); S0/K stay f32 for the log chain.
  All four outputs are f16 planes. DMA: 28 MiB -> 21 MiB per core.
- ln(S) and ln(K) are taken on RAW inputs with ACT scale=0.01
  (ln(0.01*s) in [-0.23, 0.19]) and written as float32r. The constant
  ln(100) offsets cancel in b = lnS - lnK. The small magnitude matters:
  fp32r rounds to ~12 mantissa bits (measured rel 2.3e-4), so scaling
  keeps the abs error ~5e-5 -> harmless after isv<=100 amplification.
- numer = b + (R-Q)T + 0.5*vtt is computed ON THE PE ENGINE as four
  identity-weight matmuls accumulating in PSUM (fp32r identity matmul
  measured bit-exact; mixed f32r/f16 accumulation groups work). This
  pulls the b/numer adds off the DVE wall. d2 = d1 - sv (sv = vtt*isv
  on GPSIMD) replaces the baseline's numer2 path.
- Since lnS/lnK no longer come from Sq/Kr, the discount factors are
  born f16: dq = exp(-Q*t) f16 from ACT, dr = dq^2 on GPSIMD (R = 2Q).
  The entire output side runs in f16 at DVE 2x (TT) / 4x (tensor_scalar
  for N = 0.5*erf+0.5): t1 = Sq*N1, t2 = Kr*N2, oc = t1-t2, op = oc-pc,
  odc = dr*N2, odp = dr-odc, with pc = Sq-Kr.
- exp AND ln both live in the natural_log_exp_and_others ACT table set
  (its ln measured 1.9e-5 abs err on [80,120] - fine at 2e-2 tolerance),
  so the table cycle is 2 sets (ln/exp + erf) instead of 3: ~5
  ACT_TABLE_LOADs (1283ns each) instead of 13.
- GPSIMD holds exactly 3 full-width ops (vtt, dr, sv) - 4+ measured to
  inflate DVE by ~50% in the previous session.
- Engine budget per core per tile (F=1024): DVE ~9.7us (7.5 f32-units:
  SqKr-pair mixed 2.0, d1 mixed-psum 1.0, d2 0.5, pc 0.5, N-pair 0.5,
  t1/t2/oc/odc/op/odp 0.5 each), ACT ~7.4us (lnSK-pair 2, dq 1, u 1,
  isv 1, erf-pair 2), PE ~3.5us, GPS ~6us. DVE-walled ~78us vs DMA 62us.
"""
import numpy as np

import concourse.bass as bass
import concourse.tile as tile
from concourse import bacc, mybir
from concourse.bass_utils import run_bass_kernel_spmd
from concourse.tile_rust import add_dep_helper

F32 = mybir.dt.float32
F32R = mybir.dt.float32r
F16 = mybir.dt.float16
AF = mybir.ActivationFunctionType
OP = mybir.AluOpType

R = 0.02
Q = 0.01
INV_SQRT2 = 0.7071067811865476
SCL = 0.01  # ln input scale: ln(SCL*s), offsets cancel in b

N = 8_388_608
NCORES = 8
P = 128
FD = N // NCORES // P  # 8192

_KEEP_SETS = ("natural_log_exp_and_others", "sigmoid_and_others")
_orig_get_tables = None

_NC = None
LAST_EXEC_NS = None
LAST_TRACE_DIR = None
TRACE = False


def _patch_act_tables():
    """Blank the membership of every activation-table set except the two
    we use (list order preserved so act_func_set_id indices stay valid)
    so ln/exp resolve to the combined set and erf to sigmoid_and_others."""
    global _orig_get_tables
    import concourse.hw_specs as hw_specs
    if _orig_get_tables is None:
        _orig_get_tables = hw_specs.get_activation_tables

        def patched(arch):
            tabs = _orig_get_tables(arch)
            return {
                name: (fns if name in _KEEP_SETS else set())
                for name, fns in tabs.items()
            }

        hw_specs.get_activation_tables = patched
        bacc.get_activation_tables = patched


def build_bs(FD=FD, F=1024, G=3, P=P):
    from contextlib import ExitStack
    assert FD % F == 0
    _patch_act_tables()
    ntiles = FD // F
    nchunks = F // 512  # matmul moving-operand chunks (psum bank = 512 f32)
    nc = bacc.Bacc("TRN2", target_bir_lowering=False, debug=False,
                   num_devices=NCORES)
    s_d = nc.dram_tensor("s0", [P, FD], F32, kind="ExternalInput").ap()
    k_d = nc.dram_tensor("k", [P, FD], F32, kind="ExternalInput").ap()
    t_d = nc.dram_tensor("t", [P, FD], F16, kind="ExternalInput").ap()
    v_d = nc.dram_tensor("vt", [P, FD], F16, kind="ExternalInput").ap()
    eyeP_d = nc.dram_tensor("eyep", [P, P], F32R, kind="ExternalInput").ap()
    eyeN_d = nc.dram_tensor("eyen", [P, P], F32R, kind="ExternalInput").ap()
    eyeC_d = nc.dram_tensor("eyec", [P, P], F16, kind="ExternalInput").ap()
    eyeH_d = nc.dram_tensor("eyeh", [P, P], F16, kind="ExternalInput").ap()
    oc_d = nc.dram_tensor("oc", [P, FD], F16, kind="ExternalOutput").ap()
    op_d = nc.dram_tensor("op", [P, FD], F16, kind="ExternalOutput").ap()
    odc_d = nc.dram_tensor("odc", [P, FD], F16, kind="ExternalOutput").ap()
    odp_d = nc.dram_tensor("odp", [P, FD], F16, kind="ExternalOutput").ap()

    with tile.TileContext(nc) as tc, ExitStack() as ctx:
        inp = ctx.enter_context(tc.tile_pool(name="inp", bufs=3))
        inpsk = ctx.enter_context(tc.tile_pool(name="inpsk", bufs=2))
        consts = ctx.enter_context(tc.tile_pool(name="consts", bufs=1))
        mid1 = ctx.enter_context(tc.tile_pool(name="mid1", bufs=2 * G + 1))
        mid2 = ctx.enter_context(tc.tile_pool(name="mid2", bufs=2))
        mid3 = ctx.enter_context(tc.tile_pool(name="mid3", bufs=3))
        psA = ctx.enter_context(tc.tile_pool(name="psA", bufs=2, space="PSUM"))
        mid4 = ctx.enter_context(tc.tile_pool(name="mid4", bufs=2 * G + 1))
        mid5 = ctx.enter_context(tc.tile_pool(name="mid5", bufs=2))
        mid6 = ctx.enter_context(tc.tile_pool(name="mid6", bufs=2))
        mid7 = ctx.enter_context(tc.tile_pool(name="mid7", bufs=2 * G + 1))
        outp = ctx.enter_context(tc.tile_pool(name="outp", bufs=2))

        # identity-weight matrices for the PE numer accumulation
        eyeP = consts.tile([P, P], F32R, tag="eyeP")
        nc.sync.dma_start(eyeP[:], eyeP_d)
        eyeN = consts.tile([P, P], F32R, tag="eyeN")
        nc.sync.dma_start(eyeN[:], eyeN_d)
        eyeC = consts.tile([P, P], F16, tag="eyeC")
        nc.sync.dma_start(eyeC[:], eyeC_d)
        eyeH = consts.tile([P, P], F16, tag="eyeH")
        nc.sync.dma_start(eyeH[:], eyeH_d)

        sizes = [1] + [G] * ((ntiles - 2) // G) + [1]
        assert sum(sizes) == ntiles
        glist = []
        pos = 0
        for size in sizes:
            glist.append(range(pos, pos + size))
            pos += size

        # ACT-stream phase ordering: chain every ACT op of a sub-phase after
        # all ACT ops of the previous sub-phase so the scheduler cannot
        # interleave the two table sets and thrash ACT_TABLE_LOADs.
        prev_phase = []
        cur_phase = []

        def act(*args, **kwargs):
            bi = nc.scalar.activation(*args, **kwargs)
            for p in prev_phase:
                add_dep_helper(bi.ins, p.ins, sync=False,
                               reason="act table phase ordering")
            cur_phase.append(bi)
            return bi

        def end_phase():
            if cur_phase:
                prev_phase[:] = cur_phase
                cur_phase.clear()

        # Warmup: dependency-free 8-elem exp forces the ln/exp-set
        # ACT_TABLE_LOAD during the engine preamble / input-DMA window.
        warm = mid3.tile([P, 8], F32, tag="warm", bufs=1)
        nc.vector.memset(warm[:], 0.0)
        warm2 = mid3.tile([P, 8], F32, tag="warm2", bufs=1)
        act(warm2[:], warm[:], AF.Exp)

        st = {}  # per-tile tensor handles

        def emit_tail(tiles):
            # E-phase DVE tail + output DMAs for a finished group
            for i in tiles:
                z = st.pop(i)
                sl = slice(i * F, (i + 1) * F)
                npair = mid5.tile([P, 2, F], F16, tag="npair")
                nc.vector.tensor_scalar(npair[:], z["ep"][:], scalar1=0.5,
                                        scalar2=0.5, op0=OP.mult, op1=OP.add)
                t1 = mid2.tile([P, F], F16, tag="t1")
                nc.vector.tensor_mul(t1[:], z["sq"][:], npair[:, 0])
                t2 = mid6.tile([P, F], F16, tag="t2")
                nc.vector.tensor_mul(t2[:], z["kr"][:], npair[:, 1])
                oc = outp.tile([P, F], F16, tag="oc")
                nc.vector.tensor_sub(oc[:], t1[:], t2[:])
                odc = outp.tile([P, F], F16, tag="odc")
                nc.vector.tensor_mul(odc[:], z["dqdr"][:, 1], npair[:, 1])
                op_ = outp.tile([P, F], F16, tag="op")
                nc.vector.tensor_sub(op_[:], oc[:], z["pc"][:])
                odp = outp.tile([P, F], F16, tag="odp")
                nc.vector.tensor_sub(odp[:], z["dqdr"][:, 1], odc[:])
                nc.sync.dma_start(oc_d[:, sl], oc[:])
                nc.sync.dma_start(op_d[:, sl], op_[:])
                nc.sync.dma_start(odc_d[:, sl], odc[:])
                nc.sync.dma_start(odp_d[:, sl], odp[:])

        prev_tiles = None
        for tiles in glist:
            # ---- L phase (ln/exp set) ----
            for i in tiles:
                sl = slice(i * F, (i + 1) * F)
                t = inp.tile([P, F], F16, tag="t")
                nc.sync.dma_start(t[:], t_d[:, sl])
                v = inp.tile([P, F], F16, tag="v")
                nc.sync.dma_start(v[:], v_d[:, sl])
                sk = inpsk.tile([P, 2, F], F32, tag="sk")
                nc.sync.dma_start(sk[:, 0], s_d[:, sl])
                nc.sync.dma_start(sk[:, 1], k_d[:, sl])

                # vtt on DVE: any GPSIMD op locks the shared SBUF port pair
                # and fully blocks concurrent DVE two-source ops (measured:
                # identical DVE TT ops swing 678ns -> 2.6us under GPS load).
                vtt = mid3.tile([P, F], F16, tag="vtt")
                nc.vector.tensor_mul(vtt[:], t[:], v[:])
                dqdr = mid1.tile([P, 2, F], F16, tag="dqdr")
                act(dqdr[:, 0], t[:], AF.Exp, scale=-Q)
                act(dqdr[:, 1], t[:], AF.Exp, scale=-R)
                lnsk = mid2.tile([P, 2, F], F32R, tag="lnsk")
                act(lnsk[:], sk[:], AF.Ln, scale=SCL)
                u = mid3.tile([P, F], F16, tag="u")
                act(u[:], vtt[:], AF.Ln)
                isv = mid3.tile([P, F], F16, tag="isv")
                act(isv[:], u[:], AF.Exp, scale=-0.5)
                # sv = sqrt(vtt) = vtt * isv
                sv = mid5.tile([P, F], F16, tag="sv")
                nc.vector.tensor_mul(sv[:], vtt[:], isv[:])

                # numer = lnS - lnK + (R-Q)*t + 0.5*vtt on PE (psum accum)
                numer = psA.tile([P, F], F32, tag="numer")
                for w, src, c0 in (
                    (eyeP, lnsk[:, 0], True),
                    (eyeN, lnsk[:, 1], False),
                    (eyeC, t[:], False),
                    (eyeH, vtt[:], False),
                ):
                    for c in range(nchunks):
                        cs = slice(c * 512, (c + 1) * 512)
                        nc.tensor.matmul(numer[:, cs], w[:], src[:, cs],
                                         start=c0, stop=(w is eyeH),
                                         skip_group_check=True)

                # DVE mid-chain
                sq = mid7.tile([P, F], F16, tag="sq")
                nc.vector.tensor_mul(sq[:], sk[:, 0], dqdr[:, 0])
                kr = mid4.tile([P, F], F16, tag="kr")
                nc.vector.tensor_mul(kr[:], sk[:, 1], dqdr[:, 1])
                pc = mid3.tile([P, F], F16, tag="pc", bufs=2 * G + 1)
                nc.vector.tensor_sub(pc[:], sq[:], kr[:])
                dpair = mid4.tile([P, 2, F], F16, tag="dpair")
                nc.vector.tensor_mul(dpair[:, 0], numer[:], isv[:])
                nc.vector.tensor_sub(dpair[:, 1], dpair[:, 0], sv[:])
                st[i] = dict(dqdr=dqdr, sq=sq, kr=kr, pc=pc, dpair=dpair)
            end_phase()
            # ---- E phase (erf set) for the previous group ----
            if prev_tiles is not None:
                for i in prev_tiles:
                    z = st[i]
                    ep = mid6.tile([P, 2, F], F16, tag="ep")
                    act(ep[:], z["dpair"][:], AF.Erf, scale=INV_SQRT2)
                    z["ep"] = ep
                end_phase()
                emit_tail(prev_tiles)
            prev_tiles = tiles
        # drain the last group
        for i in prev_tiles:
            z = st[i]
            ep = mid6.tile([P, 2, F], F16, tag="ep")
            act(ep[:], z["dpair"][:], AF.Erf, scale=INV_SQRT2)
            z["ep"] = ep
        end_phase()
        emit_tail(prev_tiles)
    nc.compile()
    return nc


def _get_nc():
    global _NC
    if _NC is None:
        _NC = build_bs()
    return _NC


def kernel(S0, K, T, vt):
    global LAST_EXEC_NS, LAST_TRACE_DIR
    nc = _get_nc()
    s32 = np.asarray(S0, dtype=np.float32)
    k32 = np.asarray(K, dtype=np.float32)
    t16 = np.asarray(T, dtype=np.float32).astype(np.float16)
    v16 = np.asarray(vt, dtype=np.float32).astype(np.float16)
    eye = np.eye(P, dtype=np.float32)
    eyep = eye
    eyen = -eye
    eyec = (eye * (R - Q)).astype(np.float16)
    eyeh = (eye * 0.5).astype(np.float16)
    shards = []
    for i in range(NCORES):
        sl = slice(i * P * FD, (i + 1) * P * FD)
        shards.append({
            "s0": np.ascontiguousarray(s32[sl].reshape(P, FD)),
            "k": np.ascontiguousarray(k32[sl].reshape(P, FD)),
            "t": np.ascontiguousarray(t16[sl].reshape(P, FD)),
            "vt": np.ascontiguousarray(v16[sl].reshape(P, FD)),
            "eyep": eyep, "eyen": eyen, "eyec": eyec, "eyeh": eyeh,
        })
    kwargs = {}
    if TRACE:
        import tempfile
        LAST_TRACE_DIR = tempfile.mkdtemp(prefix="bs_trace_")
        kwargs = dict(trace=True, tmpdir=LAST_TRACE_DIR)
    res = run_bass_kernel_spmd(nc, shards, core_ids=list(range(NCORES)),
                               **kwargs)
    LAST_EXEC_NS = res.exec_time_ns
    out = np.empty((N, 4), dtype=np.float32)
    for i in range(NCORES):
        sl = slice(i * P * FD, (i + 1) * P * FD)
        r = res.results[i]
        out[sl, 0] = r["oc"].reshape(-1).astype(np.float32)
        out[sl, 1] = r["op"].reshape(-1).astype(np.float32)
        out[sl, 2] = r["odc"].reshape(-1).astype(np.float32)
        out[sl, 3] = r["odp"].reshape(-1).astype(np.float32)
    return out


# revision 12
# speedup vs baseline: 1.4146x; 1.0266x over previous
"""Black-Scholes 'all' pricing on 8 Trainium2 NeuronCores (Bass/Tile).

kernel(S0, K, T, vt) -> [N, 4] float32 (call, put, digital_call, digital_put)
N = 8_388_608; options sharded contiguously across 8 cores, each core
processing 1M elements as [128 partitions x 8192] in tiles of F=1024.

Design (measured on HW across iterations; baseline 152us -> this):
- T and vt ship as f16 from the host (their error contributions are
  sqrt(vtt)-scaled through the d's, ~3e-4); S0/K stay f32 for the log
  chain. All four outputs are f16 planes. DMA: 28 MiB -> 21 MiB/core.
- GPSIMD runs NOTHING: DVE and GpSimd arbitrate for a shared SBUF port
  pair, lock-held for the full instruction - any GPS op fully blocks
  concurrent DVE two-source ops (measured: identical DVE TT ops swing
  678ns -> 2.6us under GPS load; removing GPS took exec 131us -> 110us).
- ln(S), ln(K) are taken on RAW inputs with ACT scale=0.01 and written
  as float32r (PE matmul moving dtype). fp32r rounds to ~12 mantissa
  bits, so the scaling (ln(0.01*s) in [-0.23, 0.19] vs ~4.7 unscaled)
  keeps the abs error ~5e-5; the ln(100) offsets cancel in lnS - lnK.
- numer = b + (R-Q)T + 0.5*vtt and numer2 = numer - vtt are computed ON
  THE PE ENGINE: identity-weight fp32r/f16 matmuls accumulating in PSUM
  (fp32r identity matmul measured bit-exact; mixed-dtype accumulation
  groups work; 16 matmuls + 5 weight loads per tile, PE ~30us vs its
  ~116us budget). d1 = numer*isv, d2 = numer2*isv read PSUM directly -
  PSUM operands use DVE's dedicated port (stable 1.22us, no SBUF port
  pressure).
- Discount factors are born f16 (dq = exp(-Q t), dr = exp(-R t) from
  ACT); the whole output side is f16 at DVE 2x / TSP 4x: sqkr pair mul,
  npair = 0.5*erf+0.5 (one wide 4x op), tpair = sqkr*npair, oc, pc, op,
  odc, odp. f16 TT measured 685ns, mixed 1.22us per [128,1024].
- exp AND ln live in one ACT table set (natural_log_exp_and_others; its
  ln measured 1.9e-5 abs err - fine at 2e-2 tolerance), erf in
  sigmoid_and_others: 2-set cycle, rounds sized [2,3,3] -> 6
  ACT_TABLE_LOADs (1283ns each) instead of baseline's 13.
- DMA is packed: interleaved [s|k] f32 and [t|v] f16 input pair planes,
  one [oc|op|odc|odp] f16 output plane per tile -> 3 DMAs/tile with
  4-8KB per-partition lines instead of 8 smaller ones.
- Engine budget per core: DVE ~79us (the wall), ACT ~72us, DMA ~65us,
  PE ~30us. The erf->tail dependency costs some round-boundary idle.
"""
import numpy as np

import concourse.bass as bass
import concourse.tile as tile
from concourse import bacc, mybir
from concourse.bass_utils import run_bass_kernel_spmd
from concourse.tile_rust import add_dep_helper

F32 = mybir.dt.float32
F32R = mybir.dt.float32r
F16 = mybir.dt.float16
AF = mybir.ActivationFunctionType
OP = mybir.AluOpType

R = 0.02
Q = 0.01
INV_SQRT2 = 0.7071067811865476
SCL = 0.01  # ln input scale: ln(SCL*s), offsets cancel in b

N = 8_388_608
NCORES = 8
P = 128
FD = N // NCORES // P  # 8192

_KEEP_SETS = ("natural_log_exp_and_others", "sigmoid_and_others")
_orig_get_tables = None

_NC = None
LAST_EXEC_NS = None
LAST_TRACE_DIR = None
TRACE = False


def _patch_act_tables():
    """Blank the membership of every activation-table set except the two
    we use (list order preserved so act_func_set_id indices stay valid)
    so ln/exp resolve to the combined set and erf to sigmoid_and_others."""
    global _orig_get_tables
    import concourse.hw_specs as hw_specs
    if _orig_get_tables is None:
        _orig_get_tables = hw_specs.get_activation_tables

        def patched(arch):
            tabs = _orig_get_tables(arch)
            return {
                name: (fns if name in _KEEP_SETS else set())
                for name, fns in tabs.items()
            }

        hw_specs.get_activation_tables = patched
        bacc.get_activation_tables = patched


def build_bs(FD=FD, F=1024, G=3, P=P):
    from contextlib import ExitStack
    assert FD % F == 0
    _patch_act_tables()
    ntiles = FD // F
    nchunks = F // 512  # matmul moving-operand chunks (psum bank = 512 f32)
    nc = bacc.Bacc("TRN2", target_bir_lowering=False, debug=False,
                   num_devices=NCORES)
    sk_d = nc.dram_tensor("sk", [P, 2 * FD], F32, kind="ExternalInput").ap()
    tv_d = nc.dram_tensor("tv", [P, 2 * FD], F16, kind="ExternalInput").ap()
    eyeP_d = nc.dram_tensor("eyep", [P, P], F32R, kind="ExternalInput").ap()
    eyeN_d = nc.dram_tensor("eyen", [P, P], F32R, kind="ExternalInput").ap()
    eyeC_d = nc.dram_tensor("eyec", [P, P], F16, kind="ExternalInput").ap()
    eyeH_d = nc.dram_tensor("eyeh", [P, P], F16, kind="ExternalInput").ap()
    eyeH2_d = nc.dram_tensor("eyeh2", [P, P], F16, kind="ExternalInput").ap()
    out4_d = nc.dram_tensor("out4", [P, 4 * FD], F16, kind="ExternalOutput").ap()

    with tile.TileContext(nc) as tc, ExitStack() as ctx:
        inp = ctx.enter_context(tc.tile_pool(name="inp", bufs=3))
        inpsk = ctx.enter_context(tc.tile_pool(name="inpsk", bufs=2))
        consts = ctx.enter_context(tc.tile_pool(name="consts", bufs=1))
        mid1 = ctx.enter_context(tc.tile_pool(name="mid1", bufs=2 * G + 1))
        mid2 = ctx.enter_context(tc.tile_pool(name="mid2", bufs=2))
        mid3 = ctx.enter_context(tc.tile_pool(name="mid3", bufs=3))
        psA = ctx.enter_context(tc.tile_pool(name="psA", bufs=2, space="PSUM"))
        mid4 = ctx.enter_context(tc.tile_pool(name="mid4", bufs=2 * G + 1))
        mid5 = ctx.enter_context(tc.tile_pool(name="mid5", bufs=2))
        mid6 = ctx.enter_context(tc.tile_pool(name="mid6", bufs=2))
        outp = ctx.enter_context(tc.tile_pool(name="outp", bufs=2))

        eyeP = consts.tile([P, P], F32R, tag="eyeP")
        nc.sync.dma_start(eyeP[:], eyeP_d)
        eyeN = consts.tile([P, P], F32R, tag="eyeN")
        nc.sync.dma_start(eyeN[:], eyeN_d)
        eyeC = consts.tile([P, P], F16, tag="eyeC")
        nc.sync.dma_start(eyeC[:], eyeC_d)
        eyeH = consts.tile([P, P], F16, tag="eyeH")
        nc.sync.dma_start(eyeH[:], eyeH_d)
        eyeH2 = consts.tile([P, P], F16, tag="eyeH2")
        nc.sync.dma_start(eyeH2[:], eyeH2_d)

        # round sizes: [2, G, G, ...] - 2-tile prime, G steady state
        rest = ntiles - 2
        assert rest % G == 0
        sizes = [2] + [G] * (rest // G)
        glist = []
        pos = 0
        for size in sizes:
            glist.append(range(pos, pos + size))
            pos += size

        prev_phase = []
        cur_phase = []

        def act(*args, **kwargs):
            bi = nc.scalar.activation(*args, **kwargs)
            for p in prev_phase:
                add_dep_helper(bi.ins, p.ins, sync=False,
                               reason="act table phase ordering")
            cur_phase.append(bi)
            return bi

        def end_phase():
            if cur_phase:
                prev_phase[:] = cur_phase
                cur_phase.clear()

        # Warmup: dependency-free 8-elem exp forces the ln/exp-set
        # ACT_TABLE_LOAD during the engine preamble / input-DMA window.
        warm = mid3.tile([P, 8], F32, tag="warm", bufs=1)
        nc.vector.memset(warm[:], 0.0)
        warm2 = mid3.tile([P, 8], F32, tag="warm2", bufs=1)
        act(warm2[:], warm[:], AF.Exp)

        st = {}  # per-tile tensor handles

        def emit_tail(tiles):
            # E-phase DVE tail + output DMA for a finished group
            for i in tiles:
                z = st.pop(i)
                npair = mid5.tile([P, 2, F], F16, tag="npair")
                nc.vector.tensor_scalar(npair[:], z["ep"][:], scalar1=0.5,
                                        scalar2=0.5, op0=OP.mult, op1=OP.add)
                tpair = mid6.tile([P, 2, F], F16, tag="tpair")
                nc.vector.tensor_mul(tpair[:], z["sqkr"][:], npair[:])
                out4 = outp.tile([P, 4, F], F16, tag="out4")
                nc.vector.tensor_sub(out4[:, 0], tpair[:, 0], tpair[:, 1])
                nc.vector.tensor_mul(out4[:, 2], z["dqdr"][:, 1], npair[:, 1])
                nc.vector.tensor_sub(out4[:, 1], out4[:, 0], z["pc"][:])
                nc.vector.tensor_sub(out4[:, 3], z["dqdr"][:, 1], out4[:, 2])
                nc.sync.dma_start(out4_d[:, i * 4 * F:(i + 1) * 4 * F],
                                  out4[:])

        prev_tiles = None
        for tiles in glist:
            # ---- L phase (ln/exp set) ----
            for i in tiles:
                tv = inp.tile([P, 2, F], F16, tag="tv")
                nc.sync.dma_start(tv[:], tv_d[:, i * 2 * F:(i + 1) * 2 * F])
                sk = inpsk.tile([P, 2, F], F32, tag="sk")
                nc.sync.dma_start(sk[:], sk_d[:, i * 2 * F:(i + 1) * 2 * F])

                vtt = mid3.tile([P, F], F16, tag="vtt")
                nc.vector.tensor_mul(vtt[:], tv[:, 0], tv[:, 1])
                dqdr = mid1.tile([P, 2, F], F16, tag="dqdr")
                act(dqdr[:, 0], tv[:, 0], AF.Exp, scale=-Q)
                act(dqdr[:, 1], tv[:, 0], AF.Exp, scale=-R)
                lnsk = mid2.tile([P, 2, F], F32R, tag="lnsk")
                act(lnsk[:], sk[:], AF.Ln, scale=SCL)
                u = mid3.tile([P, F], F16, tag="u")
                act(u[:], vtt[:], AF.Ln)
                isv = mid3.tile([P, F], F16, tag="isv")
                act(isv[:], u[:], AF.Exp, scale=-0.5)

                # numer / numer2 on PE: per-weight over both psum banks
                numer = psA.tile([P, F], F32, tag="numer")
                numer2 = psA.tile([P, F], F32, tag="numer2")
                for w, srcs, first, last in (
                    (eyeP, (lnsk[:, 0],), True, False),
                    (eyeN, (lnsk[:, 1],), False, False),
                    (eyeC, (tv[:, 0],), False, False),
                    (eyeH, (vtt[:],), False, True),
                    (eyeH2, (vtt[:],), False, True),
                ):
                    banks = (numer, numer2) if w not in (eyeH, eyeH2) else \
                        ((numer,) if w is eyeH else (numer2,))
                    for bank in banks:
                        for c in range(nchunks):
                            cs = slice(c * 512, (c + 1) * 512)
                            nc.tensor.matmul(bank[:, cs], w[:], srcs[0][:, cs],
                                             start=first, stop=last,
                                             skip_group_check=True)

                # DVE mid-chain
                sqkr = mid4.tile([P, 2, F], F16, tag="sqkr")
                nc.vector.tensor_mul(sqkr[:], sk[:], dqdr[:])
                pc = mid3.tile([P, F], F16, tag="pc", bufs=2 * G + 1)
                nc.vector.tensor_sub(pc[:], sqkr[:, 0], sqkr[:, 1])
                dpair = mid4.tile([P, 2, F], F16, tag="dpair")
                nc.vector.tensor_mul(dpair[:, 0], numer[:], isv[:])
                nc.vector.tensor_mul(dpair[:, 1], numer2[:], isv[:])
                st[i] = dict(dqdr=dqdr, sqkr=sqkr, pc=pc, dpair=dpair)
            end_phase()
            # ---- E phase (erf set) for the previous group ----
            if prev_tiles is not None:
                for i in prev_tiles:
                    z = st[i]
                    ep = mid6.tile([P, 2, F], F16, tag="ep")
                    act(ep[:], z["dpair"][:], AF.Erf, scale=INV_SQRT2)
                    z["ep"] = ep
                end_phase()
                emit_tail(prev_tiles)
            prev_tiles = tiles
        # drain the last group
        for i in prev_tiles:
            z = st[i]
            ep = mid6.tile([P, 2, F], F16, tag="ep")
            act(ep[:], z["dpair"][:], AF.Erf, scale=INV_SQRT2)
            z["ep"] = ep
        end_phase()
        emit_tail(prev_tiles)
    nc.compile()
    return nc


def _get_nc():
    global _NC
    if _NC is None:
        _NC = build_bs()
    return _NC


def kernel(S0, K, T, vt):
    global LAST_EXEC_NS, LAST_TRACE_DIR
    nc = _get_nc()
    F = 1024
    nt = FD // F
    s32 = np.asarray(S0, dtype=np.float32)
    k32 = np.asarray(K, dtype=np.float32)
    t16 = np.asarray(T, dtype=np.float32).astype(np.float16)
    v16 = np.asarray(vt, dtype=np.float32).astype(np.float16)
    eye = np.eye(P, dtype=np.float32)
    consts = {
        "eyep": eye, "eyen": -eye,
        "eyec": (eye * (R - Q)).astype(np.float16),
        "eyeh": (eye * 0.5).astype(np.float16),
        "eyeh2": (eye * -0.5).astype(np.float16),
    }
    shards = []
    for i in range(NCORES):
        sl = slice(i * P * FD, (i + 1) * P * FD)
        s_i = s32[sl].reshape(P, nt, F)
        k_i = k32[sl].reshape(P, nt, F)
        t_i = t16[sl].reshape(P, nt, F)
        v_i = v16[sl].reshape(P, nt, F)
        sk = np.stack([s_i, k_i], axis=2).reshape(P, 2 * FD)
        tv = np.stack([t_i, v_i], axis=2).reshape(P, 2 * FD)
        shards.append({"sk": np.ascontiguousarray(sk),
                       "tv": np.ascontiguousarray(tv), **consts})
    kwargs = {}
    if TRACE:
        import tempfile
        LAST_TRACE_DIR = tempfile.mkdtemp(prefix="bs_trace_")
        kwargs = dict(trace=True, tmpdir=LAST_TRACE_DIR)
    res = run_bass_kernel_spmd(nc, shards, core_ids=list(range(NCORES)),
                               **kwargs)
    LAST_EXEC_NS = res.exec_time_ns
    out = np.empty((N, 4), dtype=np.float32)
    for i in range(NCORES):
        sl = slice(i * P * FD, (i + 1) * P * FD)
        o4 = res.results[i]["out4"].reshape(P, nt, 4, F)
        for c in range(4):
            out[sl, c] = o4[:, :, c, :].reshape(-1).astype(np.float32)
    return out
